# revision 20
# baseline (speedup 1.0000x reference)
"""BiLSTM-CRF Trainium2 kernel: 8-core data-parallel over batch.

Self-contained: builds a Bass/Tile program once (cached), shards the batch
across 8 NeuronCores, runs SPMD, and reassembles the full [64, 256] int32
tag output.

Pipeline per core (batch 8, T=256, t-major bt = t*8+b):
  P0  constants / replicated tables (PE K=1 replicate matmuls)
  P1  pos-BiLSTM input tables + precompute + 256-step fwd/bwd recurrence
      (bwd runs in reverse time with zero-masking == packed-seq reversal)
  P2  main input precompute (char/pos-emb/pos-out contributions + bias)
      streamed to DRAM in [dir, gate-chunk, 128, bt] layout
  P3  main-BiLSTM 256-step fwd/bwd recurrence (gates on partitions,
      batch on free dim; Whh stationary on PE)
  P4  emission matmul + masked-emission trick (pads emit only STOP)
  P5  CRF Viterbi: fwd max-plus scan + bwd max-plus scan (merged per-step
      DVE ops), then position-wise argmax(alpha+beta) decode (this avoids
      the sequential backtrace entirely)
"""

import sys

sys.path.insert(0, "/opt/trn_rl_repo")

import numpy as np

import concourse.bass as bass
import concourse.tile as tile
from concourse import mybir
from concourse.bass_utils import run_bass_kernel_spmd

f32 = mybir.dt.float32
bf16 = mybir.dt.bfloat16
i32 = mybir.dt.int32

B, T = 64, 256
NCORES = 8
BPC = B // NCORES          # 8 sequences per core
BT = BPC * T               # 2048
CHAR_E, POS_E = 256, 128
POS_H, MAIN_H = 256, 512
Hp, Hm = POS_H // 2, MAIN_H // 2       # 128, 256
Dm = CHAR_E + POS_E + POS_H            # 640
LBL = 20
START, STOP = 18, 19
NEG = -1.0e9

Sigmoid = mybir.ActivationFunctionType.Sigmoid
Tanh = mybir.ActivationFunctionType.Tanh
Alu = mybir.AluOpType
AxX = mybir.AxisListType.X


# ---------------------------------------------------------------------------
# walrus compat (this container's walrus rejects Tile's stock barrier modes
# and instructions carrying >1 semaphore wait)
# ---------------------------------------------------------------------------
def _patched_multi_engine_barrier(self, engines):
    engines = list(engines)
    key = tuple(sorted(str(e) for e in engines))
    state = getattr(self, "_compat_barrier_state", None)
    if state is None:
        state = {}
        self._compat_barrier_state = state
    if key not in state:
        sem = self.alloc_semaphore(f"compat_barrier_{len(state)}")
        state[key] = [sem, 0]
    sem, count = state[key]
    state[key][1] = count + 1
    target = len(engines) * (count + 1)
    for e in engines:
        self.engines[e].drain().then_inc(sem, 1)
    for e in engines:
        self.engines[e].wait_ge(sem, target)


bass.Bass.multi_engine_barrier = _patched_multi_engine_barrier


def _split_multi_waits(nc):
    counter = [0]
    for fn in nc.m.functions:
        for bb in fn.blocks:
            out = []
            changed = False
            for ins in bb.instructions:
                si = ins.sync_info
                if si is not None and len(si.on_wait) > 1:
                    waits = list(si.on_wait)
                    for w in waits[:-1]:
                        stub = mybir.InstEventSemaphore(
                            name=f"wsplit-{counter[0]}", ins=[], outs=[]
                        )
                        counter[0] += 1
                        stub.engine = ins.engine
                        stub.sync_info = mybir.SyncInfo(on_wait=[w], on_update=[])
                        out.append(stub)
                    ins.sync_info = mybir.SyncInfo(
                        on_wait=[waits[-1]], on_update=list(si.on_update)
                    )
                    changed = True
                out.append(ins)
            if changed:
                bb.instructions = out


def _ap(t, offset_elems, dims):
    """Build an AP on tile t: partition dim kept, free dims = [[step, count]...]."""
    base = t[:]
    return bass.AP(tensor=base.tensor, offset=base.offset + offset_elems, ap=[base.ap[0]] + dims)


# ---------------------------------------------------------------------------
# kernel builder
# ---------------------------------------------------------------------------
def build(debug=False, stop=99):
    nc = bass.Bass("TRN2", target_bir_lowering=False)

    def din(name, shape, dt=f32):
        return nc.dram_tensor(name, shape, dt, kind="ExternalInput")

    charT = din("charT", [2, 128, BT])
    oneposT = din("oneposT", [31, BT])
    onetagT = din("onetagT", [31, BT])
    pos_emb_in = din("pos_emb_in", [30, 128])
    tag_pos_embT31 = din("tag_pos_embT31", [128, 31])
    pWihT = din("pWihT", [2, 128, 4 * Hp])
    pbrow = din("pbrow", [2, 1, 4 * Hp])
    pWhhT = din("pWhhT", [2, 128, 4 * Hp])
    mWihT = din("mWihT", [2, Dm, 4 * Hm])
    mbrow = din("mbrow", [2, 1, 4 * Hm])
    mWhhT = din("mWhhT", [2, Hm, 4 * Hm])
    WoutT = din("WoutT", [512, LBL])
    ones8 = din("ones8", [1, 8])
    ones128 = din("ones128", [1, 128])
    maskflat = din("maskflat", [1, BT])
    maskTb = din("maskTb", [128, 16])
    maskPadA = din("maskPadA", [128, 16, LBL])
    tablesRow = din("tablesRow", [1, 800])
    startRow = din("startRow", [1, LBL])
    stopRow = din("stopRow", [1, LBL])
    s1Row = din("s1Row", [1, LBL])
    padRow = din("padRow", [1, LBL])
    iotaRow = din("iotaRow", [1, LBL])
    maskBT = din("maskBT", [BPC, T])
    selMat = din("selMat", [2, 16])
    startStop2 = din("startStop2", [2, LBL])
    tables2 = din("tables2", [2, 400])
    wSpill = None

    out = nc.dram_tensor("out", [BPC, T], i32, kind="ExternalOutput")
    mainPre = nc.dram_tensor("mainPre", [2, 8, 128, BT], f32)
    emitSpill = nc.dram_tensor("emitSpill", [BT, LBL], f32)
    wSpill2 = nc.dram_tensor("wSpill2", [BPC, T, LBL], f32)

    dbg = {}
    if debug:
        dbg["posOut"] = nc.dram_tensor("dbg_posOut", [128, 2, T, BPC], f32, kind="ExternalOutput")
        dbg["mainOut"] = nc.dram_tensor("dbg_mainOut", [128, 2, 2, T, BPC], f32, kind="ExternalOutput")
        dbg["emitD"] = nc.dram_tensor("dbg_emitD", [BPC, T, LBL], f32, kind="ExternalOutput")
        dbg["state"] = nc.dram_tensor("dbg_state", [16, T, LBL], f32, kind="ExternalOutput")

    with tile.TileContext(nc) as tc:
        _build_body(nc, tc, locals(), stop)

    _split_multi_waits(nc)
    return nc


def _build_body(nc, tc, ios, stop=99):
    from contextlib import ExitStack

    g = dict(ios)

    with ExitStack() as ctx:
        P = ctx.enter_context(tc.tile_pool(name="G", bufs=1))
        PS = ctx.enter_context(tc.tile_pool(name="PS", bufs=2, space="PSUM"))
        PSR = ctx.enter_context(tc.tile_pool(name="PSR", bufs=2, space="PSUM"))
        W = ctx.enter_context(tc.tile_pool(name="W", bufs=3))

        # ------------- P0: load constants -------------
        t_ones8 = P.tile([1, 8], f32)
        nc.sync.dma_start(t_ones8[:], g["ones8"][:])
        t_ones128 = P.tile([1, 128], f32)
        nc.sync.dma_start(t_ones128[:], g["ones128"][:])

        def replicate(row_dram, ncols, npart, lhsT):
            """Replicate a [1, ncols] DRAM row across npart partitions."""
            t_row = W.tile([1, 2048], f32, name="repr", bufs=1)[:, :ncols]
            nc.sync.dma_start(t_row, row_dram[:])
            dst = P.tile([npart, ncols], f32, name=f"rep{row_dram.name}")
            for c0 in range(0, ncols, 512):
                n = min(512, ncols - c0)
                ps = PSR.tile([npart, 512], f32, name="psr")
                nc.tensor.matmul(ps[:, :n], lhsT, t_row[:, c0:c0 + n], start=True, stop=True)
                nc.vector.tensor_copy(dst[:, c0:c0 + n], ps[:, :n])
            return dst

        tablesRep = replicate(g["tablesRow"], 800, 8, t_ones8[:])
        startRep = replicate(g["startRow"], LBL, 8, t_ones8[:])
        stopRep = replicate(g["stopRow"], LBL, 8, t_ones8[:])
        iotaRep = replicate(g["iotaRow"], LBL, 8, t_ones8[:])
        s1Rep = replicate(g["s1Row"], LBL, 128, t_ones128[:])
        padRep = replicate(g["padRow"], LBL, 128, t_ones128[:])
        maskRep = replicate(g["maskflat"], BT, 128, t_ones128[:])

        t_maskTb = P.tile([128, 16], f32)
        nc.sync.dma_start(t_maskTb[:], g["maskTb"][:])
        t_maskPadA = P.tile([128, 16, LBL], f32)
        nc.sync.dma_start(t_maskPadA[:], g["maskPadA"][:])

        t_pWhhT = P.tile([128, 2, 512], f32)
        nc.sync.dma_start(t_pWhhT[:], g["pWhhT"][:].rearrange("d k m -> k d m"))
        t_mWhhT = P.tile([128, 2, 2, 1024], f32)
        nc.sync.dma_start(
            t_mWhhT[:], g["mWhhT"][:].rearrange("d (c k) m -> k d c m", k=128)
        )
        t_WoutT = P.tile([128, 4, LBL], f32)
        nc.sync.dma_start(t_WoutT[:], g["WoutT"][:].rearrange("(c k) m -> k c m", k=128))

        z16 = P.tile([128, 16], f32)
        nc.vector.memset(z16[:], 0.0)
        onesbf = P.tile([1, 512], f32)
        nc.vector.memset(onesbf[:], 1.0)

        posOutT = P.tile([128, 2, T, BPC], f32)

        # ------------- P1: pos-BiLSTM -------------
        with tc.tile_pool(name="PA", bufs=1) as PA:
            t_onetag = PA.tile([31, BT], f32)
            nc.sync.dma_start(t_onetag[:], g["onetagT"][:])
            t_petT = PA.tile([128, 31], f32)
            nc.sync.dma_start(t_petT[:], g["tag_pos_embT31"][:])
            t_pWihT = PA.tile([128, 2, 512], f32)
            nc.sync.dma_start(t_pWihT[:], g["pWihT"][:].rearrange("d k m -> k d m"))

            tabs = []
            for d in range(2):
                ps = PSR.tile([31, 512], f32, name="psr")
                nc.tensor.matmul(ps[:], t_petT[:], t_pWihT[:, d, :], start=True, stop=True)
                tab = PA.tile([31, 512], f32, name=f"tab{d}")
                nc.vector.tensor_copy(tab[0:30, :], ps[0:30, :])
                nc.sync.dma_start(tab[30:31, :], g["pbrow"][d, :, :])
                tabs.append(tab)

            posPre = PA.tile([128, 2, 4, BT], f32)
            for d in range(2):
                for gc in range(4):
                    for btc in range(4):
                        ps = PS.tile([128, 512], f32, name="prepsum")
                        nc.tensor.matmul(
                            ps[:],
                            tabs[d][:, gc * 128:(gc + 1) * 128],
                            t_onetag[:, btc * 512:(btc + 1) * 512],
                            start=True, stop=True,
                        )
                        eng = nc.vector if (gc + btc) % 2 == 0 else nc.scalar
                        if eng is nc.vector:
                            eng.tensor_copy(posPre[:, d, gc, btc * 512:(btc + 1) * 512], ps[:])
                        else:
                            eng.copy(posPre[:, d, gc, btc * 512:(btc + 1) * 512], ps[:])

            # recurrence
            cprev = [None, None]
            hprev_t = [None, None]
            for step in range(T):
                for d in (0, 1):
                    tt = step if d == 0 else T - 1 - step
                    psg = PSR.tile([128, 4, 8], f32, name=f"rg{d}")
                    hp = z16[:, 0:8] if step == 0 else posOutT[:, d, hprev_t[d], :]
                    for gc in range(4):
                        nc.tensor.matmul(
                            psg[:, gc, :],
                            t_pWhhT[:, d, gc * 128:(gc + 1) * 128],
                            hp, start=True, stop=True,
                        )
                    gates = W.tile([128, 4, 8], f32, name=f"pgate{d}")
                    nc.vector.tensor_add(
                        gates[:], psg[:],
                        _ap(posPre, (d * 4) * BT + tt * 8, [[BT, 4], [1, 8]]),
                    )
                    sg = W.tile([128, 3, 8], f32, name=f"psg{d}")
                    nc.scalar.activation(sg[:], gates[:, 0:3, :], Sigmoid)
                    tg = W.tile([128, 8], f32, name=f"ptg{d}")
                    nc.scalar.activation(tg[:], gates[:, 3, :], Tanh)
                    cn = W.tile([128, 8], f32, name=f"pc{d}", bufs=2)
                    if step == 0:
                        nc.vector.tensor_mul(cn[:], sg[:, 0, :], tg[:])
                    else:
                        tf = W.tile([128, 8], f32, name=f"ptf{d}")
                        nc.gpsimd.tensor_mul(tf[:], sg[:, 1, :], cprev[d][:])
                        ti = W.tile([128, 8], f32, name=f"pti{d}")
                        nc.gpsimd.tensor_mul(ti[:], sg[:, 0, :], tg[:])
                        nc.vector.tensor_add(cn[:], tf[:], ti[:])
                    tch = W.tile([128, 8], f32, name=f"ptc{d}")
                    nc.scalar.activation(tch[:], cn[:], Tanh)
                    if d == 0:
                        nc.vector.tensor_mul(posOutT[:, 0, tt, :], sg[:, 2, :], tch[:])
                        cprev[0] = cn
                    else:
                        h2 = W.tile([128, 8], f32, name="ph2")
                        nc.vector.tensor_mul(h2[:], sg[:, 2, :], tch[:])
                        m = _ap(maskRep, tt * 8, [[1, 8]])
                        nc.vector.tensor_mul(posOutT[:, 1, tt, :], h2[:], m)
                        cm = W.tile([128, 8], f32, name="pcm", bufs=2)
                        nc.vector.tensor_mul(cm[:], cn[:], m)
                        cprev[1] = cm
                    hprev_t[d] = tt

        if stop <= 1:
            return
        # ------------- P2-P4: main LSTM + emissions (scoped pool PM) -------------
        ctx_pm = ExitStack()
        PM = ctx_pm.enter_context(tc.tile_pool(name="PM", bufs=1))
        XB = ctx_pm.enter_context(tc.tile_pool(name="XB", bufs=2))
        mainOutT = PM.tile([128, 2, 2, T, BPC], f32)
        with tc.tile_pool(name="PB", bufs=1) as PB:
            t_charT = PB.tile([128, 2, BT], f32)
            nc.sync.dma_start(t_charT[:], g["charT"][:].rearrange("c k n -> k c n"))
            t_onepos = PB.tile([31, BT], f32)
            nc.sync.dma_start(t_onepos[:], g["oneposT"][:])
            t_pemb = PB.tile([30, 128], f32)
            nc.sync.dma_start(t_pemb[:], g["pos_emb_in"][:])
            posembT = PB.tile([128, BT], f32)
            for btc in range(4):
                ps = PS.tile([128, 512], f32, name="prepsum")
                nc.tensor.matmul(
                    ps[:], t_pemb[:], t_onepos[0:30, btc * 512:(btc + 1) * 512],
                    start=True, stop=True,
                )
                nc.vector.tensor_copy(posembT[:, btc * 512:(btc + 1) * 512], ps[:])

            for d in range(2):
                t_mWihT = PB.tile([128, 5, 1024], f32, name="mwih")
                nc.sync.dma_start(
                    t_mWihT[:], g["mWihT"][d, :, :].rearrange("(c k) m -> k c m", k=128)
                )
                t_mb = PB.tile([1, 1024], f32, name="mbr")
                nc.sync.dma_start(t_mb[:], g["mbrow"][d, :, :])
                for gc in range(8):
                    gs = slice(gc * 128, (gc + 1) * 128)
                    for btc in range(4):
                        bs = slice(btc * 512, (btc + 1) * 512)
                        ps = PS.tile([128, 512], f32, name="prepsum")
                        nc.tensor.matmul(ps[:], t_mWihT[:, 0, gs], t_charT[:, 0, bs], start=True, stop=False)
                        nc.tensor.matmul(ps[:], t_mWihT[:, 1, gs], t_charT[:, 1, bs], start=False, stop=False)
                        nc.tensor.matmul(ps[:], t_mWihT[:, 2, gs], posembT[:, bs], start=False, stop=False)
                        nc.tensor.matmul(
                            ps[:], t_mWihT[:, 3, gs],
                            _ap(posOutT, 0 * T * BPC + btc * 512, [[1, 512]]),
                            start=False, stop=False,
                        )
                        nc.tensor.matmul(
                            ps[:], t_mWihT[:, 4, gs],
                            _ap(posOutT, 1 * T * BPC + btc * 512, [[1, 512]]),
                            start=False, stop=False,
                        )
                        nc.tensor.matmul(
                            ps[:], t_mb[:, gs], onesbf[:],
                            start=False, stop=True,
                        )
                        bounce = PB.tile([128, 512], f32, name="mpb", bufs=3)
                        if (gc + btc) % 2 == 0:
                            nc.vector.tensor_copy(bounce[:], ps[:])
                        else:
                            nc.scalar.copy(bounce[:], ps[:])
                        nc.sync.dma_start(g["mainPre"][d, gc, :, bs], bounce[:])

        if stop <= 2:
            ctx_pm.close()
            return
        # ------------- P3: main recurrence -------------
        cprev = [None, None]
        hprev_t = [None, None]
        xblk = [None, None]
        xblk_t0 = [None, None]
        for step in range(T):
            for d in (0, 1):
                tt = step if d == 0 else T - 1 - step
                t0 = (tt // 32) * 32
                if xblk_t0[d] != t0:
                    xb = XB.tile([128, 8, 256], f32, name=f"xb{d}", bufs=2)
                    nc.sync.dma_start(
                        xb[:],
                        bass.AP(
                            tensor=g["mainPre"][:].tensor,
                            offset=g["mainPre"][:].offset + d * 8 * 128 * BT + t0 * 8,
                            ap=[[BT, 128], [128 * BT, 8], [1, 256]],
                        ),
                    )
                    xblk[d] = xb
                    xblk_t0[d] = t0
                psg = PSR.tile([128, 8, 8], f32, name=f"rg{d}")
                for gc in range(8):
                    for kc in range(2):
                        hp = (
                            z16[:, kc * 8:(kc + 1) * 8]
                            if step == 0
                            else mainOutT[:, d, kc, hprev_t[d], :]
                        )
                        nc.tensor.matmul(
                            psg[:, gc, :],
                            t_mWhhT[:, d, kc, gc * 128:(gc + 1) * 128],
                            hp, start=(kc == 0), stop=(kc == 1),
                        )
                gates = W.tile([128, 8, 8], f32, name=f"mgate{d}")
                nc.vector.tensor_add(
                    gates[:], psg[:],
                    _ap(xblk[d], (tt - xblk_t0[d]) * 8, [[256, 8], [1, 8]]),
                )
                sg = W.tile([128, 6, 8], f32, name=f"msg{d}")
                nc.scalar.activation(sg[:], gates[:, 0:6, :], Sigmoid)
                tg = W.tile([128, 2, 8], f32, name=f"mtg{d}")
                nc.scalar.activation(tg[:], gates[:, 6:8, :], Tanh)
                cn = W.tile([128, 2, 8], f32, name=f"mc{d}", bufs=2)
                if step == 0:
                    nc.vector.tensor_mul(cn[:], sg[:, 0:2, :], tg[:])
                else:
                    tf = W.tile([128, 2, 8], f32, name=f"mtf{d}")
                    nc.gpsimd.tensor_mul(tf[:], sg[:, 2:4, :], cprev[d][:])
                    ti = W.tile([128, 2, 8], f32, name=f"mti{d}")
                    nc.gpsimd.tensor_mul(ti[:], sg[:, 0:2, :], tg[:])
                    nc.vector.tensor_add(cn[:], tf[:], ti[:])
                tch = W.tile([128, 2, 8], f32, name=f"mtc{d}")
                nc.scalar.activation(tch[:], cn[:], Tanh)
                if d == 0:
                    nc.vector.tensor_mul(
                        _ap(mainOutT, 0 + tt * 8, [[T * BPC, 2], [1, 8]]),
                        sg[:, 4:6, :], tch[:],
                    )
                    cprev[0] = cn
                else:
                    h2 = W.tile([128, 2, 8], f32, name="mh2")
                    nc.vector.tensor_mul(h2[:], sg[:, 4:6, :], tch[:])
                    m2 = _ap(maskRep, tt * 8, [[0, 2], [1, 8]])
                    nc.vector.tensor_mul(
                        _ap(mainOutT, 2 * T * BPC + tt * 8, [[T * BPC, 2], [1, 8]]),
                        h2[:], m2,
                    )
                    cm = W.tile([128, 2, 8], f32, name="mcm", bufs=2)
                    nc.vector.tensor_mul(cm[:], cn[:], m2)
                    cprev[1] = cm
                hprev_t[d] = tt

        if stop <= 3:
            ctx_pm.close()
            return
        # ------------- P4: emissions -------------
        if True:
            emitA = PM.tile([128, 16, LBL], f32)
            for btc in range(16):
                ps = PSR.tile([128, LBL], f32, name="psr")
                for k in range(4):
                    nc.tensor.matmul(
                        ps[:],
                        _ap(mainOutT, k * T * BPC + btc * 128, [[1, 128]]),
                        t_WoutT[:, k, :],
                        start=(k == 0), stop=(k == 3),
                    )
                ea = W.tile([128, LBL], f32, name="ea")
                nc.vector.tensor_add(ea[:], ps[:], s1Rep[:])
                nc.vector.scalar_tensor_tensor(
                    emitA[:, btc, :], ea[:], t_maskTb[:, btc:btc + 1],
                    t_maskPadA[:, btc, :], Alu.mult, Alu.add,
                )
            nc.sync.dma_start(
                bass.AP(
                    tensor=g["emitSpill"][:].tensor,
                    offset=g["emitSpill"][:].offset,
                    ap=[[LBL, 128], [128 * LBL, 16], [1, LBL]],
                ),
                emitA[:],
            )
            if "dbg" in g and g["dbg"]:
                nc.gpsimd.dma_start(g["dbg"]["posOut"][:], posOutT[:])
                nc.gpsimd.dma_start(g["dbg"]["mainOut"][:], mainOutT[:])
            ctx_pm.close()
        # ------------- P5: viterbi (fwd+bwd folded into 16 partitions) -------------
        with tc.tile_pool(name="PC", bufs=1) as PC:
            t_sel = PC.tile([2, 16], f32)
            nc.sync.dma_start(t_sel[:], g["selMat"][:])
            t_ss2 = PC.tile([2, LBL], f32)
            nc.sync.dma_start(t_ss2[:], g["startStop2"][:])
            t_tb2 = PC.tile([2, 400], f32)
            nc.sync.dma_start(t_tb2[:], g["tables2"][:])
            tables16 = PC.tile([16, 400], f32)
            ps = PSR.tile([16, 400], f32, name="psr")
            nc.tensor.matmul(ps[:], t_sel[:], t_tb2[:], start=True, stop=True)
            nc.vector.tensor_copy(tables16[:], ps[:])
            ss16 = PC.tile([16, LBL], f32)
            ps2 = PSR.tile([16, LBL], f32, name="psr")
            nc.tensor.matmul(ps2[:], t_sel[:], t_ss2[:], start=True, stop=True)
            nc.vector.tensor_copy(ss16[:], ps2[:])

            emitD16 = PC.tile([16, T, LBL], f32)
            nc.sync.dma_start(
                emitD16[0:BPC, :, :],
                bass.AP(
                    tensor=g["emitSpill"][:].tensor,
                    offset=g["emitSpill"][:].offset,
                    ap=[[LBL, BPC], [BPC * LBL, T], [1, LBL]],
                ),
            )
            nc.sync.dma_start(
                emitD16[BPC:16, :, :],
                bass.AP(
                    tensor=g["emitSpill"][:].tensor,
                    offset=g["emitSpill"][:].offset + (T - 1) * BPC * LBL,
                    ap=[[LBL, BPC], [-BPC * LBL, T], [1, LBL]],
                ),
            )

            state = PC.tile([16, T, LBL], f32)
            nc.vector.tensor_add(state[:, 0, :], ss16[:], emitD16[:, 0, :])
            for s in range(1, T):
                scores = W.tile([16, 400], f32, name="vsc")
                nc.vector.tensor_add(
                    scores[:].rearrange("p (o i) -> p o i", o=20),
                    _ap(state, (s - 1) * LBL, [[0, 20], [1, 20]]),
                    _ap(tables16, 0, [[20, 20], [1, 20]]),
                )
                tmp = W.tile([16, LBL], f32, name="vtmp")
                nc.vector.tensor_reduce(
                    tmp[:], scores[:].rearrange("p (o i) -> p o i", o=20),
                    axis=AxX, op=Alu.max,
                )
                nc.vector.tensor_add(state[:, s, :], tmp[:], emitD16[:, s, :])

            # spill w-half (partitions 8:16) and reload time-reversed onto 0:8
            nc.sync.dma_start(g["wSpill2"][:], state[BPC:16, :, :])
            wrev = PC.tile([BPC, T, LBL], f32)
            nc.sync.dma_start(
                wrev[:],
                bass.AP(
                    tensor=g["wSpill2"][:].tensor,
                    offset=g["wSpill2"][:].offset + (T - 1) * LBL,
                    ap=[[T * LBL, BPC], [-LBL, T], [1, LBL]],
                ),
            )
            emitD = emitD16[0:BPC, :, :]

            # decode: tag_t = argmax_j(alpha_t(j) + w_rev_t(j) - emit_t(j))
            v = PC.tile([BPC, T, LBL], f32)
            nc.vector.tensor_add(v[:], state[0:BPC, :, :].rearrange("p t j -> p (t j)").rearrange("p (t j) -> p t j", j=LBL), wrev[:])
            nc.vector.tensor_sub(v[:], v[:], emitD)
            mx = PC.tile([BPC, T], f32)
            nc.vector.tensor_reduce(mx[:], v[:], axis=AxX, op=Alu.max)
            nc.vector.tensor_tensor(
                v[:], v[:], _ap(mx, 0, [[1, T], [0, LBL]]), Alu.is_equal
            )
            nc.vector.tensor_mul(v[:], v[:], _ap(iotaRep, 0, [[0, T], [1, LBL]]))
            r = PC.tile([BPC, T], f32)
            nc.vector.tensor_reduce(r[:], v[:], axis=AxX, op=Alu.max)
            t_maskBT = PC.tile([BPC, T], f32)
            nc.sync.dma_start(t_maskBT[:], g["maskBT"][:])
            nc.vector.tensor_scalar(r[:], r[:], -1.0, 19.0, Alu.mult, Alu.add)
            nc.vector.tensor_mul(r[:], r[:], t_maskBT[:])
            ti32 = PC.tile([BPC, T], i32)
            nc.vector.tensor_copy(ti32[:], r[:])
            nc.sync.dma_start(g["out"][:], ti32[:])

            if "dbg" in g and g["dbg"]:
                d = g["dbg"]
                nc.sync.dma_start(d["emitD"][:], emitD)
                nc.sync.dma_start(d["state"][:], state[:])


# ---------------------------------------------------------------------------
# host side
# ---------------------------------------------------------------------------
_CACHE = {}


def _get_nc(debug=False):
    key = ("nc", debug)
    if key not in _CACHE:
        _CACHE[key] = build(debug=debug)
    return _CACHE[key]


def _gate_perm(H):
    return np.r_[0:H, H:2 * H, 3 * H:4 * H, 2 * H:3 * H]


def _onehot31(idx_flat):
    o = np.zeros((31, idx_flat.size), np.float32)
    o[idx_flat, np.arange(idx_flat.size)] = 1.0
    o[30, :] = 1.0
    return o


try:
    from ml_dtypes import bfloat16 as np_bf16
except ImportError:
    np_bf16 = None


def prep_inputs(char_inputs, pos_inputs, tag_inputs, lengths, mask,
                char_emb, pos_emb, tag_pos_emb,
                pWih_f, pWhh_f, pb_f, pWih_b, pWhh_b, pb_b,
                mWih_f, mWhh_f, mb_f, mWih_b, mWhh_b, mb_b,
                W_out, b_out, trans):
    a32 = lambda x: np.ascontiguousarray(np.asarray(x), dtype=np.float32)
    ai = lambda x: np.asarray(x).astype(np.int64)

    char_inputs, pos_inputs, tag_inputs = ai(char_inputs), ai(pos_inputs), ai(tag_inputs)
    mask = np.asarray(mask).astype(bool)
    char_emb, pos_emb, tag_pos_emb = a32(char_emb), a32(pos_emb), a32(tag_pos_emb)
    trans = a32(trans)
    W_out, b_out = a32(W_out), a32(b_out)

    pperm = _gate_perm(Hp)
    mperm = _gate_perm(Hm)

    b16 = lambda x: np.asarray(x, dtype=np.float32)
    shared = {}
    shared["pos_emb_in"] = a32(pos_emb)
    tpe = np.zeros((128, 31), np.float32)
    tpe[:, 0:30] = tag_pos_emb.T
    shared["tag_pos_embT31"] = tpe
    shared["pWihT"] = np.stack([a32(pWih_f).T[:, pperm], a32(pWih_b).T[:, pperm]])
    shared["pbrow"] = np.stack([a32(pb_f)[pperm][None], a32(pb_b)[pperm][None]])
    shared["pWhhT"] = np.stack([a32(pWhh_f).T[:, pperm], a32(pWhh_b).T[:, pperm]])
    shared["mWihT"] = np.stack([a32(mWih_f).T[:, mperm], a32(mWih_b).T[:, mperm]])
    shared["mbrow"] = np.stack([a32(mb_f)[mperm][None], a32(mb_b)[mperm][None]])
    shared["mWhhT"] = np.stack([a32(mWhh_f).T[:, mperm], a32(mWhh_b).T[:, mperm]])
    shared["WoutT"] = a32(W_out).T
    shared["ones8"] = np.ones((1, 8), np.float32)
    shared["ones128"] = np.ones((1, 128), np.float32)
    padcol = np.full(LBL, NEG, np.float32)
    padcol[STOP] = -trans[STOP, STOP]
    shared["padRow"] = padcol[None]
    shared["s1Row"] = b_out[None]
    shared["tablesRow"] = np.concatenate([trans.T.flatten(), trans.flatten()])[None]
    shared["startRow"] = trans[START][None]
    shared["stopRow"] = trans[:, STOP][None]
    shared["iotaRow"] = (19.0 - np.arange(20, dtype=np.float32))[None]
    sel = np.zeros((2, 16), np.float32)
    sel[0, 0:8] = 1.0
    sel[1, 8:16] = 1.0
    shared["selMat"] = sel
    shared["startStop2"] = np.stack([trans[START], trans[:, STOP]])
    shared["tables2"] = np.stack(
        [trans.T.flatten(), trans.flatten()]
    )

    in_maps = []
    for c in range(NCORES):
        S = slice(c * BPC, (c + 1) * BPC)
        ci = char_inputs[S]
        m = {}
        m.update(shared)
        ct = char_emb[ci]                                  # [8, 256, 256]
        m["charT"] = np.ascontiguousarray(
            ct.transpose(2, 1, 0).reshape(2, 128, T, BPC).reshape(2, 128, BT)
        )
        m["oneposT"] = _onehot31((pos_inputs[S].T).flatten())   # t-major
        m["onetagT"] = _onehot31((tag_inputs[S].T).flatten())
        mf = mask[S].astype(np.float32)
        m["maskflat"] = np.ascontiguousarray(mf.T.flatten()[None])
        m["maskTb"] = np.ascontiguousarray(mf.T.flatten().reshape(16, 128).T)
        m["maskPadA"] = np.ascontiguousarray(
            (1.0 - m["maskTb"])[:, :, None] * padcol[None, None, :]
        )
        m["maskBT"] = np.ascontiguousarray(mf)
        in_maps.append(m)
    return in_maps


def kernel(**inputs):
    debug = bool(inputs.pop("_debug", False))
    trace = bool(inputs.pop("_trace", False))
    nc = _get_nc(debug=debug)
    in_maps = prep_inputs(**inputs)
    res = run_bass_kernel_spmd(
        nc, in_maps, core_ids=list(range(NCORES)), trace=trace
    )
    out = np.concatenate([res.results[c]["out"] for c in range(NCORES)], axis=0)
    if debug or trace:
        kernel._last = res
    return out.astype(np.int32)
